# revision 2
# baseline (speedup 1.0000x reference)
"""GGNN (JITGNN) Trainium2 kernel v2: 8-core row-parallel SpMM message passing.

Strategy (per sharding hint): shard the [N+1, N+1] adjacency row-wise across
8 cores. Each core keeps its 1000(+1 supernode on core 7) node states in
feature-major f32 layout in SBUF, projects messages each timestep, AllGathers
fp8 hi/lo messages across cores, aggregates with its fp8 adjacency shard via
DoubleRow matmuls (2x PE rate), and applies the GRU to its slice. The two
independent graphs (b, a) are interleaved so each graph's collective hides
behind the other graph's compute.

Numerics: messages quantized to fp8e4m3 hi+lo pairs (~9 significant bits),
adjacency is 0/1 (exact in fp8), aggregation accumulates in fp32 PSUM. The
supernode receives the column-sum of the quantized messages through an extra
all-ones column of core 7's adjacency shard (no separate exact path needed).
GRU gate matmuls run in float32r (tf32-class, ~1e-4 rel) with fp32
elementwise. Validated end-to-end error ~2e-3 vs fp64 reference (tolerance
2e-2). Final 2-class head on host in fp64.
"""

import numpy as np
import ml_dtypes

try:
    import concourse.bacc  # noqa: F401
except ImportError:  # pragma: no cover
    import sys

    sys.path.insert(0, "/opt/trn_rl_repo")

F8 = ml_dtypes.float8_e4m3
HIDDEN = 256
N = 8000
NC = 8             # cores
SLOT = 1024        # padded node slots per core (1000 real, +1 supernode on core 7)
REAL = N // NC     # 1000 real rows per core
JTOT = NC * SLOT   # 8192 padded message rows
ACH = 8            # adjacency chunks per graph: [128, 8 ktile, 1024]


def _prep_adj_shards(adj):
    """adj [8000,8000] 0/1 fp32 -> per-core chunks [ACH, 128, 8, 1024] fp8.

    Contraction slot j' = 1024*d + 8*p + a in AllGather-row order maps to
    source slot u_src = a*128 + p on core d (messages are written node-major
    with partition p = u%128, a = u//128, giving contiguous 2KB DMA rows).
    In (ktile, lane) coordinates kt = d*8 + a, lane = p, the j'-slot is
    d*1024 + a*128 + p -- natural order. Chunk g8 covers kt in [8g8, 8g8+8),
    i.e. exactly source core d = g8.

    A_dram[g8, p, ktl, u] = A_aug[i(u,c), j(d=g8, u_src=ktl*128+p)].
    Core 7 column u=1000 (supernode) is 1 for every real source slot.
    """
    AT = np.ascontiguousarray(adj.T).astype(F8)              # [j, i]
    Jmat = np.zeros((JTOT, N), dtype=F8)
    for d in range(NC):
        Jmat[SLOT * d : SLOT * d + REAL] = AT[REAL * d : REAL * (d + 1)]
    shards = []
    for c in range(NC):
        R = np.zeros((JTOT, SLOT), dtype=F8)
        R[:, :REAL] = Jmat[:, REAL * c : REAL * (c + 1)]
        if c == NC - 1:
            # supernode receives colsum of all real-node messages
            sup = np.zeros((JTOT,), dtype=F8)
            for d in range(NC):
                sup[SLOT * d : SLOT * d + REAL] = 1.0
            R[:, REAL] = sup
        chunks = np.ascontiguousarray(
            R.reshape(ACH, 8, 128, SLOT).transpose(0, 2, 1, 3)
        )  # [g8, p, ktl, u]
        shards.append(chunks)
    return shards


def _prep_h0_shards(x):
    """x [8000, 256] fp32 -> per-core feature-major state [2, 128, SLOT] f32."""
    xT = x.T.astype(np.float32)  # [256, 8000]
    shards = []
    for c in range(NC):
        H = np.zeros((HIDDEN, SLOT), dtype=np.float32)
        H[:, :REAL] = xT[:, REAL * c : REAL * (c + 1)]
        shards.append(np.ascontiguousarray(H.reshape(2, 128, SLOT)))
    return shards


def _pack_w(w_t, cols):
    """w.T [256, cols] -> [128, 2, cols] f32 with (partition, kt, out-feature)."""
    return np.ascontiguousarray(
        w_t.astype(np.float32).reshape(2, 128, cols).transpose(1, 0, 2)
    )


def _build_program(T):
    import concourse.bacc as bacc
    import concourse.mybir as mybir
    from concourse import tile

    f8 = mybir.dt.float8e4
    f32 = mybir.dt.float32
    f32r = mybir.dt.float32r
    Alu = mybir.AluOpType
    Act = mybir.ActivationFunctionType
    DR = mybir.MatmulPerfMode.DoubleRow

    nc = bacc.Bacc("TRN2", target_bir_lowering=False, debug=False, num_devices=NC)

    GR = ("b", "a")
    A_in = {g: nc.dram_tensor(f"A_{g}", [ACH, 128, 8, SLOT], f8, kind="ExternalInput") for g in GR}
    H0_in = {g: nc.dram_tensor(f"h0_{g}", [2, 128, SLOT], f32r, kind="ExternalInput") for g in GR}
    Wlin_in = nc.dram_tensor("Wlin", [128, 2, 256], f32r, kind="ExternalInput")
    Wih_in = nc.dram_tensor("Wih", [128, 2, 768], f32r, kind="ExternalInput")
    Whh_in = nc.dram_tensor("Whh", [128, 2, 768], f32r, kind="ExternalInput")
    Blin_in = nc.dram_tensor("Blin", [128, 256], f32, kind="ExternalInput")
    Brz_in = nc.dram_tensor("Brz", [128, 4], f32, kind="ExternalInput")
    Bin_in = nc.dram_tensor("Bin", [128, 2], f32, kind="ExternalInput")
    Bhn_in = nc.dram_tensor("Bhn", [128, 2], f32, kind="ExternalInput")
    HO_out = {g: nc.dram_tensor(f"ho_{g}", [2, 128, 1], f32r, kind="ExternalOutput") for g in GR}

    rg = [list(range(NC))]

    with tile.TileContext(nc) as tc:
        with (
            tc.tile_pool(name="const", bufs=1) as constp,
            tc.tile_pool(name="a_stream", bufs=3) as a_pool,
            tc.tile_pool(name="lhs_stream", bufs=3) as lhs_pool,
            tc.tile_pool(name="state", bufs=2) as state_pool,
            tc.tile_pool(name="mpool", bufs=1) as m_pool,
            tc.tile_pool(name="msgs", bufs=1) as msgs_pool,
            tc.tile_pool(name="mf", bufs=2) as mf_pool,
            tc.tile_pool(name="rz", bufs=4) as rz_pool,
            tc.tile_pool(name="tmp", bufs=4) as tmp_pool,
            tc.tile_pool(name="psA", bufs=2, space="PSUM") as psum_agg,
            tc.tile_pool(name="psG", bufs=2, space="PSUM") as psum_gates,
            tc.tile_pool(name="dram", bufs=2, space="DRAM") as dram_pool,
        ):
            # ---- constants ----
            wlin = constp.tile([128, 2, 256], f32r, name="wlin")
            nc.sync.dma_start(wlin[:], Wlin_in[:])
            wih = constp.tile([128, 2, 768], f32r, name="wih")
            nc.sync.dma_start(wih[:], Wih_in[:])
            whh = constp.tile([128, 2, 768], f32r, name="whh")
            nc.sync.dma_start(whh[:], Whh_in[:])
            blin = constp.tile([128, 256], f32, name="blin")
            nc.sync.dma_start(blin[:], Blin_in[:])
            brz = constp.tile([128, 4], f32, name="brz")
            nc.sync.dma_start(brz[:], Brz_in[:])
            bin_ = constp.tile([128, 2], f32, name="bin_")
            nc.sync.dma_start(bin_[:], Bin_in[:])
            bhn = constp.tile([128, 2], f32, name="bhn")
            nc.sync.dma_start(bhn[:], Bhn_in[:])

            # ---- state load ----
            H = {}
            for g in GR:
                H[g] = []
                for kt in range(2):
                    h = state_pool.tile([128, SLOT], f32r, name=f"h_{g}{kt}", tag=f"h_{g}{kt}")
                    nc.sync.dma_start(h[:], H0_in[g][kt, :, :])
                    H[g].append(h)

            # graph b's adjacency stays resident in SBUF (64KB/partition);
            # graph a's is streamed per timestep, halving steady DMA traffic.
            # Loaded after the state/constant DMAs so the first messages
            # matmul isn't queued behind 8.4MB of adjacency transfer.
            a_res = constp.tile([128, ACH * 8, SLOT], f8, name="a_res")
            for g8 in range(ACH):
                nc.sync.dma_start(a_res[:, g8 * 8 : (g8 + 1) * 8, :], A_in["b"][g8, :, :, :])

            cc_out = {}

            def emit_msgs_allgather(g):
                """msgs (node-major) -> fp8 hi/lo -> cc_in -> AllGather."""
                msgs8 = msgs_pool.tile([128, 2, 8, 256], f8, name=f"msgs8_{g}", tag=f"msgs8_{g}")
                psMs = [
                    psum_gates.tile([128, 1024], f32, name=f"psm_{g}{i}", tag="psG")
                    for i in range(2)
                ]
                for a in range(8):
                    reg = psMs[a // 4][:, (a % 4) * 256 : (a % 4 + 1) * 256]
                    for kt in range(2):
                        nc.tensor.matmul(
                            reg,
                            lhsT=H[g][kt][:, a * 128 : (a + 1) * 128],
                            rhs=wlin[:, kt, :],
                            start=(kt == 0),
                            stop=(kt == 1),
                        )
                    mf = mf_pool.tile([128, 256], f32, name=f"mf_{g}{a}", tag="mf")
                    nc.vector.tensor_add(mf[:], reg, blin[:])
                    nc.vector.tensor_copy(msgs8[:, 0, a, :], mf[:])
                    nc.vector.tensor_sub(msgs8[:, 1, a, :], mf[:], msgs8[:, 0, a, :])
                cc_in = dram_pool.tile([2 * SLOT, 256], f8, name=f"cc_in_{g}", tag=f"cc_in_{g}")
                nc.sync.dma_start(
                    cc_in[:].rearrange("(b p a) f -> p b a f", b=2, p=128),
                    msgs8[:],
                )
                cco = dram_pool.tile(
                    [2 * JTOT, 256], f8, name=f"cc_out_{g}", tag=f"cc_out_{g}", addr_space="Shared"
                )
                nc.gpsimd.collective_compute(
                    "AllGather",
                    mybir.AluOpType.bypass,
                    replica_groups=rg,
                    ins=[cc_in.opt()],
                    outs=[cco.opt()],
                )
                cc_out[g] = cco

            def emit_agg(g):
                """m.T [256, SLOT] = sum_j msgs[j] A[j, u] via fp8 DoubleRow."""
                psA = [
                    psum_agg.tile([128, SLOT], f32, name=f"psA_{g}{mi}", tag="psA")
                    for mi in range(2)
                ]
                for g8 in range(ACH):
                    if g == "b":
                        at_base, at_off = a_res, g8 * 8
                    else:
                        at_t = a_pool.tile([128, 8, SLOT], f8, name=f"at_{g}{g8}", tag="at")
                        nc.sync.dma_start(at_t[:], A_in[g][g8, :, :, :])
                        at_base, at_off = at_t, 0
                    lh = lhs_pool.tile([128, 2, 8, 256], f8, name=f"lh_{g}{g8}", tag="lh")
                    nc.sync.dma_start(
                        lh[:],
                        cc_out[g][2 * SLOT * g8 : 2 * SLOT * (g8 + 1), :].rearrange(
                            "(b p a) f -> p b a f", b=2, p=128
                        ),
                    )
                    for qq in range(4):
                        for mi in range(2):
                            for ni in range(2):
                                reg = psA[mi][:, ni * 512 : (ni + 1) * 512]
                                rhs = at_base[
                                    :,
                                    at_off + 2 * qq : at_off + 2 * qq + 2,
                                    ni * 512 : (ni + 1) * 512,
                                ]
                                nc.tensor.matmul(
                                    reg,
                                    lhsT=lh[:, 0, 2 * qq : 2 * qq + 2, mi * 128 : (mi + 1) * 128],
                                    rhs=rhs,
                                    start=(g8 == 0 and qq == 0),
                                    stop=False,
                                    perf_mode=DR,
                                )
                                nc.tensor.matmul(
                                    reg,
                                    lhsT=lh[:, 1, 2 * qq : 2 * qq + 2, mi * 128 : (mi + 1) * 128],
                                    rhs=rhs,
                                    start=False,
                                    stop=(g8 == ACH - 1 and qq == 3),
                                    perf_mode=DR,
                                )
                m = []
                for mi in range(2):
                    mt = m_pool.tile([128, SLOT], f32r, name=f"m_{g}{mi}", tag=f"m_{g}{mi}")
                    nc.vector.tensor_copy(mt[:], psA[mi][:])
                    m.append(mt)
                return m

            def emit_gru(g, m):
                """Gate matmuls (f32r) + fp32 elementwise GRU update of H[g]."""
                old_H = list(H[g])

                def fused_gate(G, name):
                    ps = psum_gates.tile([128, 1024], f32, name=name, tag="psG")
                    for ni in range(2):
                        n_mm = 0
                        for kt in range(2):
                            for w, r in ((wih, m), (whh, old_H)):
                                nc.tensor.matmul(
                                    ps[:, ni * 512 : (ni + 1) * 512],
                                    lhsT=w[:, kt, G * 128 : (G + 1) * 128],
                                    rhs=r[kt][:, ni * 512 : (ni + 1) * 512],
                                    start=(n_mm == 0),
                                    stop=(n_mm == 3),
                                )
                                n_mm += 1
                    return ps

                def half_gate(G, w, r, name):
                    ps = psum_gates.tile([128, 1024], f32, name=name, tag="psG")
                    for ni in range(2):
                        for kt in range(2):
                            nc.tensor.matmul(
                                ps[:, ni * 512 : (ni + 1) * 512],
                                lhsT=w[:, kt, G * 128 : (G + 1) * 128],
                                rhs=r[kt][:, ni * 512 : (ni + 1) * 512],
                                start=(kt == 0),
                                stop=(kt == 1),
                            )
                    return ps

                rr, zz = [], []
                for ch in range(2):
                    ps = fused_gate(ch, f"ps_r{g}{ch}")
                    r_t = rz_pool.tile([128, SLOT], f32, name=f"r_{g}{ch}", tag="rz")
                    nc.scalar.activation(r_t[:], ps[:], Act.Sigmoid, bias=brz[:, ch : ch + 1])
                    rr.append(r_t)
                for ch in range(2):
                    ps = fused_gate(2 + ch, f"ps_z{g}{ch}")
                    z_t = rz_pool.tile([128, SLOT], f32, name=f"z_{g}{ch}", tag="rz")
                    nc.scalar.activation(z_t[:], ps[:], Act.Sigmoid, bias=brz[:, 2 + ch : 3 + ch])
                    zz.append(z_t)

                for ch in range(2):
                    ps_i = half_gate(4 + ch, wih, m, f"ps_i{g}{ch}")
                    ps_h = half_gate(4 + ch, whh, old_H, f"ps_h{g}{ch}")
                    t1 = tmp_pool.tile([128, SLOT], f32, name=f"t1_{g}{ch}", tag="tmp")
                    nc.vector.scalar_tensor_tensor(
                        t1[:], ps_h[:], bhn[:, ch : ch + 1], rr[ch][:], Alu.add, Alu.mult
                    )
                    t2 = tmp_pool.tile([128, SLOT], f32, name=f"t2_{g}{ch}", tag="tmp")
                    nc.vector.tensor_add(t2[:], t1[:], ps_i[:])
                    n_t = tmp_pool.tile([128, SLOT], f32, name=f"n_{g}{ch}", tag="tmp")
                    nc.scalar.activation(n_t[:], t2[:], Act.Tanh, bias=bin_[:, ch : ch + 1])
                    d_t = tmp_pool.tile([128, SLOT], f32, name=f"d_{g}{ch}", tag="tmp")
                    nc.vector.tensor_sub(d_t[:], old_H[ch][:], n_t[:])
                    t3 = tmp_pool.tile([128, SLOT], f32, name=f"t3_{g}{ch}", tag="tmp")
                    nc.vector.tensor_mul(t3[:], zz[ch][:], d_t[:])
                    h_new = state_pool.tile([128, SLOT], f32r, name=f"h_{g}{ch}", tag=f"h_{g}{ch}")
                    nc.vector.tensor_add(h_new[:], n_t[:], t3[:])
                    H[g][ch] = h_new

            if T >= 1:
                for g in GR:
                    emit_msgs_allgather(g)
                for t in range(T):
                    for g in GR:
                        m = emit_agg(g)
                        emit_gru(g, m)
                        if t < T - 1:
                            emit_msgs_allgather(g)

            for g in GR:
                for kt in range(2):
                    nc.sync.dma_start(HO_out[g][kt, :, :], H[g][kt][:, REAL : REAL + 1])

    nc.compile()
    return nc


def prepare(inputs):
    """Build+compile the program and the per-core input maps.

    Returns (nc, in_maps, postprocess) where postprocess maps core 7's
    result dict to the final [2] log-softmax output.
    """
    b_x = np.asarray(inputs["b_x"], dtype=np.float32)
    a_x = np.asarray(inputs["a_x"], dtype=np.float32)
    b_adj = np.asarray(inputs["b_adj"], dtype=np.float32)
    a_adj = np.asarray(inputs["a_adj"], dtype=np.float32)
    W_lin = np.asarray(inputs["W_lin"], dtype=np.float32)
    b_lin = np.asarray(inputs["b_lin"], dtype=np.float32)
    W_ih = np.asarray(inputs["W_ih"], dtype=np.float32)
    b_ih = np.asarray(inputs["b_ih"], dtype=np.float32)
    W_hh = np.asarray(inputs["W_hh"], dtype=np.float32)
    b_hh = np.asarray(inputs["b_hh"], dtype=np.float32)
    W_fc = np.asarray(inputs["W_fc"], dtype=np.float32)
    b_fc = np.asarray(inputs["b_fc"], dtype=np.float32)
    T = int(inputs["n_timesteps"])

    nc = _build_program(T)

    A_shards = {"b": _prep_adj_shards(b_adj), "a": _prep_adj_shards(a_adj)}
    H0_shards = {"b": _prep_h0_shards(b_x), "a": _prep_h0_shards(a_x)}
    wlin_p = _pack_w(W_lin.T, 256)
    wih_p = _pack_w(W_ih.T, 768)
    whh_p = _pack_w(W_hh.T, 768)
    blin_b = np.ascontiguousarray(np.broadcast_to(b_lin.astype(np.float32), (128, 256)))
    brz_p = np.ascontiguousarray((b_ih[:512] + b_hh[:512]).astype(np.float32).reshape(4, 128).T)
    bin_p = np.ascontiguousarray(b_ih[512:768].astype(np.float32).reshape(2, 128).T)
    bhn_p = np.ascontiguousarray(b_hh[512:768].astype(np.float32).reshape(2, 128).T)

    in_maps = []
    for c in range(NC):
        in_maps.append(
            {
                "A_b": A_shards["b"][c],
                "A_a": A_shards["a"][c],
                "h0_b": H0_shards["b"][c],
                "h0_a": H0_shards["a"][c],
                "Wlin": wlin_p,
                "Wih": wih_p,
                "Whh": whh_p,
                "Blin": blin_b,
                "Brz": brz_p,
                "Bin": bin_p,
                "Bhn": bhn_p,
            }
        )

    def post(out7):
        sup = np.concatenate(
            [
                np.asarray(out7["ho_b"]).reshape(HIDDEN),
                np.asarray(out7["ho_a"]).reshape(HIDDEN),
            ]
        ).astype(np.float64)
        logits = sup @ W_fc.astype(np.float64).T + b_fc.astype(np.float64)
        mx = logits.max()
        return (logits - mx - np.log(np.exp(logits - mx).sum())).astype(np.float32)

    return nc, in_maps, post


def run(inputs, trace=False):
    from concourse.bass_utils import run_bass_kernel_spmd

    nc, in_maps, post = prepare(inputs)
    res = run_bass_kernel_spmd(nc, in_maps, core_ids=list(range(NC)), trace=trace)
    return post(res.results[NC - 1]), res.exec_time_ns


def kernel(**inputs):
    out, _ = run(inputs, trace=False)
    return out
